# revision 1
# baseline (speedup 1.0000x reference)
"""Trainium2 Bass kernel for a fused GRU cell.

Reference computation (B=4096, IN=1024, H=1024, all fp32):
    x_proj = x @ W_ih.T + b_ih            # (B, 3H)
    r_x, z_x, n_x = split(x_proj, 3)
    rz_h = h @ W_rzh.T                    # (B, 2H)
    r = sigmoid(r_x + r_h); z = sigmoid(z_x + z_h)
    n = tanh(n_x + r * (h @ W_nh.T + b_nh))
    out = (1-z)*n + z*h

Strategy:
  - Data-parallel over batch across 8 NeuronCores (512 rows each);
    weights replicated (packed host-side into PE-friendly tiles).
  - Transposed layout on chip: features on partitions, batch on the free
    dim, so per-feature biases are per-partition ACT activation biases.
  - r/z projections fused into ONE K=2048 contraction by concatenating
    [x;h] and [W_ih[:2H].T; W_rzh.T] host-side.
  - Matmuls in fp16 (1 cycle/row on PE, 2 bytes of HBM traffic) with
    fp32 PSUM accumulation; everything else fp32.
"""

import numpy as np

import concourse.mybir as mybir
import concourse.tile as tile
from concourse import bacc
from concourse.bass_utils import run_bass_kernel_spmd

B, IN, H = 4096, 1024, 1024
NCORES = 8
BC = B // NCORES          # 512 batch rows per core
P = 128

KO_RZ = (IN + H) // P     # 16 contraction subtiles for the fused r/z matmul
G_RZ = 2 * H // P         # 16 gate tiles (0..7 = r, 8..15 = z)
KO_N = IN // P            # 8
G_N = H // P              # 8

F16 = mybir.dt.float16
F32 = mybir.dt.float32
AF = mybir.ActivationFunctionType
ALU = mybir.AluOpType


def build_bass():
    """Build the per-core Bass program (identical on all cores)."""
    nc = bacc.Bacc("TRN2", target_bir_lowering=False, debug=False)

    xh_d = nc.dram_tensor("xh", [P, KO_RZ, BC], F16, kind="ExternalInput")
    hf_d = nc.dram_tensor("hf", [P, G_N, BC], F32, kind="ExternalInput")
    wrz_d = nc.dram_tensor("wrz", [G_RZ, P, KO_RZ, P], F16, kind="ExternalInput")
    wnx_d = nc.dram_tensor("wnx", [G_N, P, KO_N, P], F16, kind="ExternalInput")
    wnh_d = nc.dram_tensor("wnh", [G_N, P, KO_N, P], F16, kind="ExternalInput")
    brz_d = nc.dram_tensor("brz", [P, G_RZ], F32, kind="ExternalInput")
    bn_d = nc.dram_tensor("bn", [P, G_N], F32, kind="ExternalInput")
    bnh_d = nc.dram_tensor("bnh", [P, G_N], F32, kind="ExternalInput")
    out_d = nc.dram_tensor("outp", [P, G_N, BC], F32, kind="ExternalOutput")

    with tile.TileContext(nc) as tc:
        with (
            tc.tile_pool(name="const", bufs=1) as cpool,
            tc.tile_pool(name="wrzp", bufs=4) as wrzp,
            tc.tile_pool(name="wnp", bufs=3) as wnp,
            tc.tile_pool(name="rzp", bufs=1) as rzp,
            tc.tile_pool(name="tmp", bufs=4) as tp,
            tc.tile_pool(name="ps_rz", bufs=3, space="PSUM") as pp_rz,
            tc.tile_pool(name="ps_x", bufs=2, space="PSUM") as pp_x,
            tc.tile_pool(name="ps_h", bufs=2, space="PSUM") as pp_h,
            tc.tile_pool(name="ps_w", bufs=1, space="PSUM") as pp_w,
        ):
            # Pre-warm the PE clock (HAM gates it to 1.2 GHz until ~3.4us
            # of sustained activity): dummy matmuls on memset scratch run
            # during the DMA-wait window before the first real weights
            # arrive, so the real stream starts at the full 2.4 GHz.
            wa = cpool.tile([P, P], F16, tag="warm_l")
            nc.vector.memset(wa[:], 0.0)
            wb = cpool.tile([P, BC], F16, tag="warm_r")
            nc.vector.memset(wb[:], 0.0)
            ps_warm = pp_w.tile([P, BC], F32, tag="warm_ps")
            for _ in range(28):
                nc.tensor.matmul(ps_warm[:], wa[:], wb[:], start=True, stop=True)
            # DMA issue order matters: transfers complete roughly in issue
            # order, and the first matmul needs only the first ko-chunk of
            # the g=0 weight tile plus the first xh chunk. Those two go
            # first, split into sub-tile DMAs (deps are view-overlap-based)
            # and routed via the gpsimd/SWDGE queue so they aren't stuck
            # behind the bulk HWDGE stream.
            w0 = wrzp.tile([P, KO_RZ, P], F16, tag="wrz")
            nc.gpsimd.dma_start(out=w0[:, 0:4, :], in_=wrz_d[0, :, 0:4, :])
            XH_CH = 4
            xh_chunks = []
            for c in range(KO_RZ // XH_CH):
                t = cpool.tile([P, XH_CH, BC], F16, tag=f"xh{c}", name=f"xh{c}")
                if c == 0:
                    nc.gpsimd.dma_start(out=t[:, 0:2, :], in_=xh_d[:, 0:2, :])
                    nc.sync.dma_start(out=w0[:, 4:, :], in_=wrz_d[0, :, 4:, :])
                    nc.sync.dma_start(out=t[:, 2:4, :], in_=xh_d[:, 2:4, :])
                else:
                    nc.sync.dma_start(
                        out=t[:], in_=xh_d[:, c * XH_CH:(c + 1) * XH_CH, :]
                    )
                xh_chunks.append(t)
            xh_sb = [
                xh_chunks[ko // XH_CH][:, ko % XH_CH, :] for ko in range(KO_RZ)
            ]
            brz_sb = cpool.tile([P, G_RZ], F32, tag="brz")
            nc.sync.dma_start(out=brz_sb[:], in_=brz_d[:])

            bn_sb = cpool.tile([P, G_N], F32, tag="bn")
            bnh_sb = cpool.tile([P, G_N], F32, tag="bnh")

            # Fused r/z projection (16 gate tiles x K=2048), with the
            # n-gate/output-blend work for tile j interleaved after r/z
            # tile 8+j: the serial DVE chain (t -> tanh -> blend) then
            # starts mid-stream and hides under the remaining matmuls
            # instead of pacing a trailing phase of its own.
            rz_blk = rzp.tile([P, G_RZ, BC], F32, tag="rzblk")
            omz_blk = rzp.tile([P, G_N, BC], F32, tag="omzblk")
            zh_blk = rzp.tile([P, G_N, BC], F32, tag="zhblk")
            hf_sb = rzp.tile([P, G_N, BC], F32, tag="hfblk")
            HB = BC // 2  # elementwise half-batch granularity
            for g in range(G_RZ):
                if g == 0:
                    w = w0
                else:
                    w = wrzp.tile([P, KO_RZ, P], F16, tag="wrz")
                    nc.sync.dma_start(out=w[:], in_=wrz_d[g])
                if g == 4 or g == 6:
                    # fp32 h halves, needed from the z tiles (g >= 8) onward
                    half = (g - 4) // 2
                    nc.sync.dma_start(
                        out=hf_sb[:, half * 4:(half + 1) * 4, :],
                        in_=hf_d[:, half * 4:(half + 1) * 4, :],
                    )
                if g == 6:
                    nc.sync.dma_start(out=bn_sb[:], in_=bn_d[:])
                    nc.sync.dma_start(out=bnh_sb[:], in_=bnh_d[:])
                ps = pp_rz.tile([P, BC], F32, tag="psrz")
                for ko in range(KO_RZ):
                    nc.tensor.matmul(
                        ps[:], w[:, ko, :], xh_sb[ko],
                        start=(ko == 0), stop=(ko == KO_RZ - 1),
                    )
                rz = rz_blk[:, g, :]
                nc.scalar.activation(
                    rz, ps[:], AF.Sigmoid, bias=brz_sb[:, g:g + 1]
                )
                if g < G_N:
                    continue
                # ---- n gate + blend for output tile j = g - 8 ----
                j = g - G_N
                nc.vector.tensor_scalar(
                    omz_blk[:, j, :], rz, -1.0, 1.0, op0=ALU.mult, op1=ALU.add
                )
                nc.vector.tensor_mul(
                    out=zh_blk[:, j, :], in0=rz, in1=hf_sb[:, j, :]
                )
                wh = wnp.tile([P, KO_N, P], F16, tag="wnh")
                nc.sync.dma_start(out=wh[:], in_=wnh_d[j])
                wx = wnp.tile([P, KO_N, P], F16, tag="wnx")
                nc.sync.dma_start(out=wx[:], in_=wnx_d[j])
                psx = pp_x.tile([P, BC], F32, tag="psx")
                psh = pp_h.tile([P, BC], F32, tag="psh")
                for ko in range(KO_N):
                    nc.tensor.matmul(
                        psh[:], wh[:, ko, :], xh_sb[KO_N + ko],
                        start=(ko == 0), stop=(ko == KO_N - 1),
                    )
                for ko in range(KO_N):
                    nc.tensor.matmul(
                        psx[:], wx[:, ko, :], xh_sb[ko],
                        start=(ko == 0), stop=(ko == KO_N - 1),
                    )
                o = tp.tile([P, BC], F32, tag="o")
                # Final tile: quarter-batch chunks so the post-matmul
                # serial chain (add -> tanh -> blend -> store) is half as
                # long on the kernel's critical tail.
                n_chunks = 4 if j == G_N - 1 else 2
                CH = BC // n_chunks
                for hb in range(n_chunks):
                    s = slice(hb * CH, (hb + 1) * CH)
                    # t = (psh + b_nh) * r    (overlaps the psx matmuls)
                    t = tp.tile([P, CH], F32, tag=f"t{hb}")
                    nc.vector.scalar_tensor_tensor(
                        t[:], psh[:, s], bnh_sb[:, j:j + 1], rz_blk[:, j, s],
                        op0=ALU.add, op1=ALU.mult,
                    )
                    nc.vector.tensor_add(out=t[:], in0=t[:], in1=psx[:, s])
                    # n = tanh(t + b_n)
                    n_t = tp.tile([P, CH], F32, tag=f"n{hb}")
                    nc.scalar.activation(
                        n_t[:], t[:], AF.Tanh, bias=bn_sb[:, j:j + 1]
                    )
                    # out = n*(1-z) + z*h
                    u = tp.tile([P, CH], F32, tag=f"u{hb}")
                    nc.vector.tensor_mul(
                        out=u[:], in0=n_t[:], in1=omz_blk[:, j, s]
                    )
                    nc.vector.tensor_add(
                        out=o[:, s], in0=u[:], in1=zh_blk[:, j, s]
                    )
                    nc.sync.dma_start(out=out_d[:, j, s], in_=o[:, s])

    nc.compile()
    return nc


def prepare_inputs(x, h, W_ih, b_ih, W_rzh, W_nh, b_nh):
    """Host-side packing: shard batch, transpose/concat/cast weights."""
    f16 = np.float16
    # Fused r/z weight: (IN+H, 2H) -> [g, p, ko, mi] tile-major
    wrz_cat = np.concatenate(
        [W_ih[: 2 * H].T, W_rzh.T], axis=0
    ).astype(f16)
    wrz = np.ascontiguousarray(
        wrz_cat.reshape(KO_RZ, P, G_RZ, P).transpose(2, 1, 0, 3)
    )
    wnx = np.ascontiguousarray(
        W_ih[2 * H:].T.astype(f16).reshape(KO_N, P, G_N, P).transpose(2, 1, 0, 3)
    )
    wnh = np.ascontiguousarray(
        W_nh.T.astype(f16).reshape(KO_N, P, G_N, P).transpose(2, 1, 0, 3)
    )
    brz = np.ascontiguousarray(b_ih[: 2 * H].reshape(G_RZ, P).T).astype(np.float32)
    bn = np.ascontiguousarray(b_ih[2 * H:].reshape(G_N, P).T).astype(np.float32)
    bnh = np.ascontiguousarray(b_nh.reshape(G_N, P).T).astype(np.float32)

    xh_catT = np.concatenate([x.T, h.T], axis=0).astype(f16)  # (2048, B)
    hT = np.ascontiguousarray(h.T.astype(np.float32))          # (1024, B)

    in_maps = []
    for c in range(NCORES):
        cols = slice(c * BC, (c + 1) * BC)
        xh_c = np.ascontiguousarray(
            xh_catT[:, cols].reshape(KO_RZ, P, BC).transpose(1, 0, 2)
        )
        hf_c = np.ascontiguousarray(
            hT[:, cols].reshape(G_N, P, BC).transpose(1, 0, 2)
        )
        in_maps.append(
            {
                "xh": xh_c,
                "hf": hf_c,
                "wrz": wrz,
                "wnx": wnx,
                "wnh": wnh,
                "brz": brz,
                "bn": bn,
                "bnh": bnh,
            }
        )
    return in_maps


def assemble_output(results):
    """results: list of per-core dicts with 'outp' [P, G_N, BC] fp32."""
    parts = []
    for c in range(NCORES):
        oc = results[c]["outp"]                       # [128, 8, 512]
        ocT = oc.transpose(1, 0, 2).reshape(H, BC)    # features x batch
        parts.append(np.ascontiguousarray(ocT.T))     # batch x features
    return np.concatenate(parts, axis=0).astype(np.float32)


def kernel(x, h, W_ih, b_ih, W_rzh, W_nh, b_nh):
    x = np.asarray(x, dtype=np.float32)
    h = np.asarray(h, dtype=np.float32)
    W_ih = np.asarray(W_ih, dtype=np.float32)
    b_ih = np.asarray(b_ih, dtype=np.float32)
    W_rzh = np.asarray(W_rzh, dtype=np.float32)
    W_nh = np.asarray(W_nh, dtype=np.float32)
    b_nh = np.asarray(b_nh, dtype=np.float32)

    in_maps = prepare_inputs(x, h, W_ih, b_ih, W_rzh, W_nh, b_nh)
    nc = build_bass()
    res = run_bass_kernel_spmd(nc, in_maps, core_ids=list(range(NCORES)))
    return assemble_output(res.results)



# revision 3
# speedup vs baseline: 1.3876x; 1.3876x over previous
"""Trainium2 Bass kernel for a fused GRU cell — fp8 DoubleRow edition.

Reference computation (B=4096, IN=1024, H=1024, all fp32):
    x_proj = x @ W_ih.T + b_ih            # (B, 3H)
    r_x, z_x, n_x = split(x_proj, 3)
    rz_h = h @ W_rzh.T                    # (B, 2H)
    r = sigmoid(r_x + r_h); z = sigmoid(z_x + z_h)
    n = tanh(n_x + r * (h @ W_nh.T + b_nh))
    out = (1-z)*n + z*h

Strategy:
  - Data-parallel over batch across 8 NeuronCores (512 rows each);
    weights replicated (packed host-side into PE-friendly tiles).
  - Transposed layout on chip: features on partitions, batch on the free
    dim, so per-feature biases are per-partition ACT activation biases.
  - r/z projections fused into ONE K=2048 contraction by concatenating
    [x;h] and [W_ih[:2H].T; W_rzh.T] host-side.
  - Matmuls in fp8 e4m3 with perf_mode=DoubleRow: each PE instruction
    contracts K=256 (two interleaved fp8 weights per cell), ~1.8x the
    bf16 MACs/cycle.  Weights pre-scaled by 256 so they sit in e4m3's
    normal range; the 1/256 is folded into the ACT scale (sigmoid/tanh)
    and b_nh is pre-scaled by 256 so the whole n-gate chain runs in the
    x256 domain until the tanh.
  - PSUM accumulation fp32; elementwise intermediates (r/z/omz/zh/h/out)
    fp16 for 2x DVE throughput; 1-z and z*h offloaded to gpsimd.
"""

import numpy as np
import ml_dtypes

import concourse.mybir as mybir
import concourse.tile as tile
from concourse import bacc
from concourse.bass_utils import run_bass_kernel_spmd

B, IN, H = 4096, 1024, 1024
NCORES = 8
BC = B // NCORES          # 512 batch rows per core
P = 128

KO_RZ = (IN + H) // P     # 16 contraction subtiles for the fused r/z matmul
G_RZ = 2 * H // P         # 16 gate tiles (0..7 = r, 8..15 = z)
KO_N = IN // P            # 8
G_N = H // P              # 8
WS = 256.0                # fp8 weight pre-scale

F8 = mybir.dt.float8e4
F16 = mybir.dt.float16
F32 = mybir.dt.float32
AF = mybir.ActivationFunctionType
ALU = mybir.AluOpType
DR = mybir.MatmulPerfMode.DoubleRow
NP8 = ml_dtypes.float8_e4m3


def build_bass():
    """Build the per-core Bass program (identical on all cores)."""
    nc = bacc.Bacc("TRN2", target_bir_lowering=False, debug=False)

    xh_d = nc.dram_tensor("xh", [P, KO_RZ, BC], F8, kind="ExternalInput")
    hf_d = nc.dram_tensor("hf", [P, G_N, BC], F16, kind="ExternalInput")
    wrz_d = nc.dram_tensor("wrz", [G_RZ, P, KO_RZ, P], F8, kind="ExternalInput")
    wn_d = nc.dram_tensor("wn", [G_N, P, 2, KO_N, P], F8, kind="ExternalInput")
    brz_d = nc.dram_tensor("brz", [P, G_RZ], F32, kind="ExternalInput")
    bn_d = nc.dram_tensor("bn", [P, G_N], F32, kind="ExternalInput")
    bnh_d = nc.dram_tensor("bnh", [P, G_N], F32, kind="ExternalInput")
    out_d = nc.dram_tensor("outp", [P, G_N, BC], F16, kind="ExternalOutput")

    with tile.TileContext(nc) as tc:
        with (
            tc.tile_pool(name="const", bufs=1) as cpool,
            tc.tile_pool(name="wrzp", bufs=4) as wrzp,
            tc.tile_pool(name="wnp", bufs=3) as wnp,
            tc.tile_pool(name="rzp", bufs=1) as rzp,
            tc.tile_pool(name="tmp", bufs=4) as tp,
            tc.tile_pool(name="ps_rz", bufs=3, space="PSUM") as pp_rz,
            tc.tile_pool(name="ps_x", bufs=2, space="PSUM") as pp_x,
            tc.tile_pool(name="ps_h", bufs=2, space="PSUM") as pp_h,
            tc.tile_pool(name="ps_w", bufs=1, space="PSUM") as pp_w,
        ):
            # HAM warm-up: the PE clock sits at 1.2 GHz until ~3.4us of
            # sustained activity.  Small N=128 dummy matmuls fill the
            # preamble/DMA-wait window so the real stream starts at 2.4 GHz.
            wa = cpool.tile([P, P], F16, tag="warm_l")
            nc.vector.memset(wa[:], 0.0)
            wb = cpool.tile([P, P], F16, tag="warm_r")
            nc.vector.memset(wb[:], 0.0)
            ps_warm = pp_w.tile([P, P], F32, tag="warm_ps")
            for _ in range(32):
                nc.tensor.matmul(ps_warm[:], wa[:], wb[:], start=True, stop=True)

            # First-use DMAs go first on the sync/HWDGE queue, split so the
            # very first matmul's operands (w0 ko0-1, xh ko0-1) land ASAP.
            w0 = wrzp.tile([P, KO_RZ, P], F8, tag="wrz")
            nc.sync.dma_start(out=w0[:, 0:2, :], in_=wrz_d[0, :, 0:2, :])
            XH_CH = 4
            xh_chunks = []
            for c in range(KO_RZ // XH_CH):
                t = cpool.tile([P, XH_CH, BC], F8, tag=f"xh{c}", name=f"xh{c}")
                if c == 0:
                    nc.sync.dma_start(out=t[:, 0:2, :], in_=xh_d[:, 0:2, :])
                    nc.sync.dma_start(out=w0[:, 2:, :], in_=wrz_d[0, :, 2:, :])
                    nc.sync.dma_start(out=t[:, 2:4, :], in_=xh_d[:, 2:4, :])
                else:
                    nc.sync.dma_start(
                        out=t[:], in_=xh_d[:, c * XH_CH:(c + 1) * XH_CH, :]
                    )
                xh_chunks.append(t)

            def xh_pair(kk):  # [P, 2, BC] fp8 view for DoubleRow pair kk
                c, i = divmod(2 * kk, XH_CH)
                return xh_chunks[c][:, i:i + 2, :]

            brz_sb = cpool.tile([P, G_RZ], F32, tag="brz")
            nc.sync.dma_start(out=brz_sb[:], in_=brz_d[:])
            bn_sb = cpool.tile([P, G_N], F32, tag="bn")
            nc.scalar.dma_start(out=bn_sb[:], in_=bn_d[:])
            bnh_sb = cpool.tile([P, G_N], F32, tag="bnh")
            nc.scalar.dma_start(out=bnh_sb[:], in_=bnh_d[:])

            rz_blk = rzp.tile([P, G_RZ, BC], F16, tag="rzblk")
            omz_blk = rzp.tile([P, G_N, BC], F16, tag="omzblk")
            zh_blk = rzp.tile([P, G_N, BC], F16, tag="zhblk")
            hf_sb = rzp.tile([P, G_N, BC], F16, tag="hfblk")

            for g in range(G_RZ):
                if g == 0:
                    w = w0
                else:
                    w = wrzp.tile([P, KO_RZ, P], F8, tag="wrz")
                    nc.sync.dma_start(out=w[:], in_=wrz_d[g])
                if g == 4 or g == 6:
                    # fp16 h halves, needed from the z tiles (g >= 8) onward
                    half = (g - 4) // 2
                    nc.gpsimd.dma_start(
                        out=hf_sb[:, half * 4:(half + 1) * 4, :],
                        in_=hf_d[:, half * 4:(half + 1) * 4, :],
                    )
                ps = pp_rz.tile([P, BC], F32, tag="psrz")
                for kk in range(KO_RZ // 2):
                    nc.tensor.matmul(
                        ps[:], w[:, 2 * kk:2 * kk + 2, :], xh_pair(kk),
                        start=(kk == 0), stop=(kk == KO_RZ // 2 - 1),
                        perf_mode=DR,
                    )
                rz = rz_blk[:, g, :]
                nc.scalar.activation(
                    rz, ps[:], AF.Sigmoid, bias=brz_sb[:, g:g + 1], scale=1.0 / WS
                )
                if g < G_N:
                    continue
                # ---- n gate + blend for output tile j = g - 8 ----
                j = g - G_N
                nc.gpsimd.tensor_scalar(
                    omz_blk[:, j, :], rz, -1.0, 1.0, op0=ALU.mult, op1=ALU.add
                )
                nc.gpsimd.tensor_mul(
                    out=zh_blk[:, j, :], in0=rz, in1=hf_sb[:, j, :]
                )
                wn_t = wnp.tile([P, 2, KO_N, P], F8, tag="wn")
                nc.gpsimd.dma_start(out=wn_t[:], in_=wn_d[j])
                psx = pp_x.tile([P, BC], F32, tag="psx")
                psh = pp_h.tile([P, BC], F32, tag="psh")
                for kk in range(KO_N // 2):
                    nc.tensor.matmul(
                        psh[:], wn_t[:, 0, 2 * kk:2 * kk + 2, :],
                        xh_pair(KO_N // 2 + kk),
                        start=(kk == 0), stop=(kk == KO_N // 2 - 1),
                        perf_mode=DR,
                    )
                for kk in range(KO_N // 2):
                    nc.tensor.matmul(
                        psx[:], wn_t[:, 1, 2 * kk:2 * kk + 2, :], xh_pair(kk),
                        start=(kk == 0), stop=(kk == KO_N // 2 - 1),
                        perf_mode=DR,
                    )
                o = tp.tile([P, BC], F16, tag="o")
                # Final tile: quarter-batch chunks so the post-matmul
                # serial chain (add -> tanh -> blend -> store) is short
                # on the kernel's critical tail.
                n_chunks = 4 if j == G_N - 1 else 2
                CH = BC // n_chunks
                for hb in range(n_chunks):
                    s = slice(hb * CH, (hb + 1) * CH)
                    # t = (psh + 256*b_nh) * r    (x256 domain)
                    t = tp.tile([P, CH], F32, tag=f"t{hb}")
                    nc.vector.scalar_tensor_tensor(
                        t[:], psh[:, s], bnh_sb[:, j:j + 1], rz_blk[:, j, s],
                        op0=ALU.add, op1=ALU.mult,
                    )
                    nc.vector.tensor_add(out=t[:], in0=t[:], in1=psx[:, s])
                    # n = tanh(t/256 + b_n)
                    n_t = tp.tile([P, CH], F16, tag=f"n{hb}")
                    nc.scalar.activation(
                        n_t[:], t[:], AF.Tanh, bias=bn_sb[:, j:j + 1],
                        scale=1.0 / WS,
                    )
                    # out = n*(1-z) + z*h
                    u = tp.tile([P, CH], F16, tag=f"u{hb}")
                    nc.vector.tensor_mul(
                        out=u[:], in0=n_t[:], in1=omz_blk[:, j, s]
                    )
                    nc.vector.tensor_add(
                        out=o[:, s], in0=u[:], in1=zh_blk[:, j, s]
                    )
                    nc.scalar.dma_start(out=out_d[:, j, s], in_=o[:, s])

    nc.compile()
    return nc


def q8(a):
    return np.clip(a, -240.0, 240.0).astype(NP8)


def prepare_inputs(x, h, W_ih, b_ih, W_rzh, W_nh, b_nh):
    """Host-side packing: shard batch, transpose/concat/quantize weights."""
    # Fused r/z weight: (IN+H, 2H) -> [g, p, ko, mi] tile-major, fp8 x256
    wrz_cat = q8(np.concatenate([W_ih[: 2 * H].T, W_rzh.T], axis=0) * WS)
    wrz = np.ascontiguousarray(
        wrz_cat.reshape(KO_RZ, P, G_RZ, P).transpose(2, 1, 0, 3)
    )
    wnx = q8(W_ih[2 * H:].T * WS).reshape(KO_N, P, G_N, P).transpose(2, 1, 0, 3)
    wnh = q8(W_nh.T * WS).reshape(KO_N, P, G_N, P).transpose(2, 1, 0, 3)
    # [G, P, 2, KO, P]: slot 0 = W_nh, slot 1 = W_nx
    wn = np.ascontiguousarray(np.stack([wnh, wnx], axis=2))
    brz = np.ascontiguousarray(b_ih[: 2 * H].reshape(G_RZ, P).T).astype(np.float32)
    bn = np.ascontiguousarray(b_ih[2 * H:].reshape(G_N, P).T).astype(np.float32)
    bnh = np.ascontiguousarray((b_nh * WS).reshape(G_N, P).T).astype(np.float32)

    xh_catT = q8(np.concatenate([x.T, h.T], axis=0))           # (2048, B) fp8
    hT = np.ascontiguousarray(h.T.astype(np.float16))          # (1024, B)

    in_maps = []
    for c in range(NCORES):
        cols = slice(c * BC, (c + 1) * BC)
        xh_c = np.ascontiguousarray(
            xh_catT[:, cols].reshape(KO_RZ, P, BC).transpose(1, 0, 2)
        )
        hf_c = np.ascontiguousarray(
            hT[:, cols].reshape(G_N, P, BC).transpose(1, 0, 2)
        ).view(np.float16)
        in_maps.append(
            {
                "xh": xh_c,
                "hf": hf_c,
                "wrz": wrz,
                "wn": wn,
                "brz": brz,
                "bn": bn,
                "bnh": bnh,
            }
        )
    return in_maps


def assemble_output(results):
    """results: list of per-core dicts with 'outp' [P, G_N, BC] fp16."""
    parts = []
    for c in range(NCORES):
        oc = np.asarray(results[c]["outp"]).astype(np.float32)
        ocT = oc.transpose(1, 0, 2).reshape(H, BC)    # features x batch
        parts.append(np.ascontiguousarray(ocT.T))     # batch x features
    return np.concatenate(parts, axis=0).astype(np.float32)


def kernel(x, h, W_ih, b_ih, W_rzh, W_nh, b_nh):
    x = np.asarray(x, dtype=np.float32)
    h = np.asarray(h, dtype=np.float32)
    W_ih = np.asarray(W_ih, dtype=np.float32)
    b_ih = np.asarray(b_ih, dtype=np.float32)
    W_rzh = np.asarray(W_rzh, dtype=np.float32)
    W_nh = np.asarray(W_nh, dtype=np.float32)
    b_nh = np.asarray(b_nh, dtype=np.float32)

    in_maps = prepare_inputs(x, h, W_ih, b_ih, W_rzh, W_nh, b_nh)
    nc = build_bass()
    res = run_bass_kernel_spmd(nc, in_maps, core_ids=list(range(NCORES)))
    return assemble_output(res.results)


# revision 4
# speedup vs baseline: 1.4097x; 1.0159x over previous
"""Trainium2 Bass kernel for a fused GRU cell — fp8 DoubleRow edition.

Reference computation (B=4096, IN=1024, H=1024, all fp32):
    x_proj = x @ W_ih.T + b_ih            # (B, 3H)
    r_x, z_x, n_x = split(x_proj, 3)
    rz_h = h @ W_rzh.T                    # (B, 2H)
    r = sigmoid(r_x + r_h); z = sigmoid(z_x + z_h)
    n = tanh(n_x + r * (h @ W_nh.T + b_nh))
    out = (1-z)*n + z*h

Strategy:
  - Data-parallel over batch across 8 NeuronCores (512 rows each);
    weights replicated (packed host-side into PE-friendly tiles).
  - Transposed layout on chip: features on partitions, batch on the free
    dim, so per-feature biases are per-partition ACT activation biases.
  - r/z projections fused into ONE K=2048 contraction by concatenating
    [x;h] and [W_ih[:2H].T; W_rzh.T] host-side.
  - Matmuls in fp8 e4m3 with perf_mode=DoubleRow: each PE instruction
    contracts K=256 (two interleaved fp8 weights per cell), hitting the
    fp8 roofline (~216ns per N=512 instruction).  Weights pre-scaled by
    256 so they sit in e4m3's normal range; the 1/256 is folded into the
    ACT scale (sigmoid/tanh) and b_nh is pre-scaled by 256 so the whole
    n-gate chain stays in the x256 domain until the tanh.
  - PSUM accumulation fp32; elementwise intermediates (r/z/omz/zh/h/out)
    fp16 for 2x DVE throughput.
  - DMA issue parallelized across queues (sync: weights + first xh;
    scalar: remaining xh; gpsimd: n-weights/h/biases; sync again for
    output stores) — serial descriptor-issue (~0.6-0.8us each) on a
    single queue starved the early matmul stream otherwise.
"""

import numpy as np
import ml_dtypes

import concourse.mybir as mybir
import concourse.tile as tile
from concourse import bacc
from concourse.bass_utils import run_bass_kernel_spmd

B, IN, H = 4096, 1024, 1024
NCORES = 8
BC = B // NCORES          # 512 batch rows per core
P = 128

KO_RZ = (IN + H) // P     # 16 contraction subtiles for the fused r/z matmul
G_RZ = 2 * H // P         # 16 gate tiles (0..7 = r, 8..15 = z)
KO_N = IN // P            # 8
G_N = H // P              # 8
WS = 256.0                # fp8 weight pre-scale

F8 = mybir.dt.float8e4
F16 = mybir.dt.float16
F32 = mybir.dt.float32
AF = mybir.ActivationFunctionType
ALU = mybir.AluOpType
DR = mybir.MatmulPerfMode.DoubleRow
NP8 = ml_dtypes.float8_e4m3


def build_bass():
    """Build the per-core Bass program (identical on all cores)."""
    nc = bacc.Bacc("TRN2", target_bir_lowering=False, debug=False)

    xh_d = nc.dram_tensor("xh", [P, KO_RZ, BC], F8, kind="ExternalInput")
    hf_d = nc.dram_tensor("hf", [P, G_N, BC], F16, kind="ExternalInput")
    wrz_d = nc.dram_tensor("wrz", [G_RZ, P, KO_RZ, P], F8, kind="ExternalInput")
    wn_d = nc.dram_tensor("wn", [G_N, P, 2, KO_N, P], F8, kind="ExternalInput")
    brz_d = nc.dram_tensor("brz", [P, G_RZ], F32, kind="ExternalInput")
    bn_d = nc.dram_tensor("bn", [P, G_N], F32, kind="ExternalInput")
    bnh_d = nc.dram_tensor("bnh", [P, G_N], F32, kind="ExternalInput")
    out_d = nc.dram_tensor("outp", [P, G_N, BC], F16, kind="ExternalOutput")

    with tile.TileContext(nc) as tc:
        with (
            tc.tile_pool(name="const", bufs=1) as cpool,
            tc.tile_pool(name="wrzp", bufs=4) as wrzp,
            tc.tile_pool(name="wnp", bufs=3) as wnp,
            tc.tile_pool(name="rzp", bufs=1) as rzp,
            tc.tile_pool(name="tmp", bufs=4) as tp,
            tc.tile_pool(name="ps_rz", bufs=3, space="PSUM") as pp_rz,
            tc.tile_pool(name="ps_x", bufs=2, space="PSUM") as pp_x,
            tc.tile_pool(name="ps_h", bufs=2, space="PSUM") as pp_h,
            tc.tile_pool(name="ps_w", bufs=1, space="PSUM") as pp_w,
        ):
            # HAM warm-up: the PE clock sits at 1.2 GHz until ~3.4us of
            # sustained activity.  Small N=128 dummy matmuls fill the
            # preamble/DMA-wait window so the real stream starts at 2.4 GHz.
            wa = cpool.tile([P, P], F16, tag="warm_l")
            nc.vector.memset(wa[:], 0.0)
            wb = cpool.tile([P, P], F16, tag="warm_r")
            nc.vector.memset(wb[:], 0.0)
            ps_warm = pp_w.tile([P, P], F32, tag="warm_ps")
            for _ in range(32):
                nc.tensor.matmul(ps_warm[:], wa[:], wb[:], start=True, stop=True)

            # First-use DMAs: sync carries gate-0 weights + the first xh
            # chunk (split so the very first matmul's operands land ASAP),
            # then the remaining r/z gate weights.  The scalar queue
            # concurrently issues the other xh chunks; gpsimd carries the
            # n-gate weights, fp16 h and biases.
            w0 = wrzp.tile([P, KO_RZ, P], F8, tag="wrz")
            nc.sync.dma_start(out=w0[:, 0:2, :], in_=wrz_d[0, :, 0:2, :])
            XH_CH = 4
            xh_chunks = []
            for c in range(KO_RZ // XH_CH):
                t = cpool.tile([P, XH_CH, BC], F8, tag=f"xh{c}", name=f"xh{c}")
                if c == 0:
                    nc.sync.dma_start(out=t[:, 0:2, :], in_=xh_d[:, 0:2, :])
                    nc.sync.dma_start(out=w0[:, 2:, :], in_=wrz_d[0, :, 2:, :])
                    nc.sync.dma_start(out=t[:, 2:4, :], in_=xh_d[:, 2:4, :])
                else:
                    nc.scalar.dma_start(
                        out=t[:], in_=xh_d[:, c * XH_CH:(c + 1) * XH_CH, :]
                    )
                xh_chunks.append(t)

            def xh_pair(kk):  # [P, 2, BC] fp8 view for DoubleRow pair kk
                c, i = divmod(2 * kk, XH_CH)
                return xh_chunks[c][:, i:i + 2, :]

            brz_sb = cpool.tile([P, G_RZ], F32, tag="brz")
            nc.scalar.dma_start(out=brz_sb[:], in_=brz_d[:])
            bn_sb = cpool.tile([P, G_N], F32, tag="bn")
            nc.gpsimd.dma_start(out=bn_sb[:], in_=bn_d[:])
            bnh_sb = cpool.tile([P, G_N], F32, tag="bnh")
            nc.gpsimd.dma_start(out=bnh_sb[:], in_=bnh_d[:])

            rz_blk = rzp.tile([P, G_RZ, BC], F16, tag="rzblk")
            omz_blk = rzp.tile([P, G_N, BC], F16, tag="omzblk")
            zh_blk = rzp.tile([P, G_N, BC], F16, tag="zhblk")
            hf_sb = rzp.tile([P, G_N, BC], F16, tag="hfblk")

            for g in range(G_RZ):
                if g == 0:
                    w = w0
                else:
                    w = wrzp.tile([P, KO_RZ, P], F8, tag="wrz")
                    nc.sync.dma_start(out=w[:], in_=wrz_d[g])
                if g == 2 or g == 4:
                    # fp16 h halves, needed from the z tiles (g >= 8) onward
                    half = (g - 2) // 2
                    nc.gpsimd.dma_start(
                        out=hf_sb[:, half * 4:(half + 1) * 4, :],
                        in_=hf_d[:, half * 4:(half + 1) * 4, :],
                    )
                ps = pp_rz.tile([P, BC], F32, tag="psrz")
                for kk in range(KO_RZ // 2):
                    nc.tensor.matmul(
                        ps[:], w[:, 2 * kk:2 * kk + 2, :], xh_pair(kk),
                        start=(kk == 0), stop=(kk == KO_RZ // 2 - 1),
                        perf_mode=DR,
                    )
                rz = rz_blk[:, g, :]
                nc.scalar.activation(
                    rz, ps[:], AF.Sigmoid, bias=brz_sb[:, g:g + 1], scale=1.0 / WS
                )
                if g < G_N:
                    continue
                # ---- n gate + blend for output tile j = g - 8 ----
                j = g - G_N
                nc.vector.tensor_scalar(
                    omz_blk[:, j, :], rz, -1.0, 1.0, op0=ALU.mult, op1=ALU.add
                )
                nc.vector.tensor_mul(
                    out=zh_blk[:, j, :], in0=rz, in1=hf_sb[:, j, :]
                )
                wn_t = wnp.tile([P, 2, KO_N, P], F8, tag="wn")
                nc.gpsimd.dma_start(out=wn_t[:], in_=wn_d[j])
                psx = pp_x.tile([P, BC], F32, tag="psx")
                psh = pp_h.tile([P, BC], F32, tag="psh")
                for kk in range(KO_N // 2):
                    nc.tensor.matmul(
                        psh[:], wn_t[:, 0, 2 * kk:2 * kk + 2, :],
                        xh_pair(KO_N // 2 + kk),
                        start=(kk == 0), stop=(kk == KO_N // 2 - 1),
                        perf_mode=DR,
                    )
                n_chunks = 4 if j == G_N - 1 else 2
                CH = BC // n_chunks
                # t = (psh + 256*b_nh) * r  (x256 domain) — depends only on
                # psh, so it overlaps the psx matmul group below.
                ts = []
                for hb in range(n_chunks):
                    s = slice(hb * CH, (hb + 1) * CH)
                    t = tp.tile([P, CH], F32, tag=f"t{hb}")
                    nc.vector.scalar_tensor_tensor(
                        t[:], psh[:, s], bnh_sb[:, j:j + 1], rz_blk[:, j, s],
                        op0=ALU.add, op1=ALU.mult,
                    )
                    ts.append(t)
                for kk in range(KO_N // 2):
                    nc.tensor.matmul(
                        psx[:], wn_t[:, 1, 2 * kk:2 * kk + 2, :], xh_pair(kk),
                        start=(kk == 0), stop=(kk == KO_N // 2 - 1),
                        perf_mode=DR,
                    )
                o = tp.tile([P, BC], F16, tag="o")
                for hb in range(n_chunks):
                    s = slice(hb * CH, (hb + 1) * CH)
                    t = ts[hb]
                    nc.vector.tensor_add(out=t[:], in0=t[:], in1=psx[:, s])
                    # n = tanh(t/256 + b_n)
                    n_t = tp.tile([P, CH], F16, tag=f"n{hb}")
                    nc.scalar.activation(
                        n_t[:], t[:], AF.Tanh, bias=bn_sb[:, j:j + 1],
                        scale=1.0 / WS,
                    )
                    # out = n*(1-z) + z*h
                    u = tp.tile([P, CH], F16, tag=f"u{hb}")
                    nc.vector.tensor_mul(
                        out=u[:], in0=n_t[:], in1=omz_blk[:, j, s]
                    )
                    nc.vector.tensor_add(
                        out=o[:, s], in0=u[:], in1=zh_blk[:, j, s]
                    )
                    if n_chunks > 2:
                        nc.sync.dma_start(out=out_d[:, j, s], in_=o[:, s])
                if n_chunks <= 2:
                    nc.sync.dma_start(out=out_d[:, j, :], in_=o[:])

    nc.compile()
    return nc


def q8(a):
    return np.clip(a, -240.0, 240.0).astype(NP8)


def prepare_inputs(x, h, W_ih, b_ih, W_rzh, W_nh, b_nh):
    """Host-side packing: shard batch, transpose/concat/quantize weights."""
    # Fused r/z weight: (IN+H, 2H) -> [g, p, ko, mi] tile-major, fp8 x256
    wrz_cat = q8(np.concatenate([W_ih[: 2 * H].T, W_rzh.T], axis=0) * WS)
    wrz = np.ascontiguousarray(
        wrz_cat.reshape(KO_RZ, P, G_RZ, P).transpose(2, 1, 0, 3)
    )
    wnx = q8(W_ih[2 * H:].T * WS).reshape(KO_N, P, G_N, P).transpose(2, 1, 0, 3)
    wnh = q8(W_nh.T * WS).reshape(KO_N, P, G_N, P).transpose(2, 1, 0, 3)
    # [G, P, 2, KO, P]: slot 0 = W_nh, slot 1 = W_nx
    wn = np.ascontiguousarray(np.stack([wnh, wnx], axis=2))
    brz = np.ascontiguousarray(b_ih[: 2 * H].reshape(G_RZ, P).T).astype(np.float32)
    bn = np.ascontiguousarray(b_ih[2 * H:].reshape(G_N, P).T).astype(np.float32)
    bnh = np.ascontiguousarray((b_nh * WS).reshape(G_N, P).T).astype(np.float32)

    xh_catT = q8(np.concatenate([x.T, h.T], axis=0))           # (2048, B) fp8
    hT = np.ascontiguousarray(h.T.astype(np.float16))          # (1024, B)

    in_maps = []
    for c in range(NCORES):
        cols = slice(c * BC, (c + 1) * BC)
        xh_c = np.ascontiguousarray(
            xh_catT[:, cols].reshape(KO_RZ, P, BC).transpose(1, 0, 2)
        )
        hf_c = np.ascontiguousarray(
            hT[:, cols].reshape(G_N, P, BC).transpose(1, 0, 2)
        )
        in_maps.append(
            {
                "xh": xh_c,
                "hf": hf_c,
                "wrz": wrz,
                "wn": wn,
                "brz": brz,
                "bn": bn,
                "bnh": bnh,
            }
        )
    return in_maps


def assemble_output(results):
    """results: list of per-core dicts with 'outp' [P, G_N, BC] fp16."""
    parts = []
    for c in range(NCORES):
        oc = np.asarray(results[c]["outp"]).astype(np.float32)
        ocT = oc.transpose(1, 0, 2).reshape(H, BC)    # features x batch
        parts.append(np.ascontiguousarray(ocT.T))     # batch x features
    return np.concatenate(parts, axis=0).astype(np.float32)


def kernel(x, h, W_ih, b_ih, W_rzh, W_nh, b_nh):
    x = np.asarray(x, dtype=np.float32)
    h = np.asarray(h, dtype=np.float32)
    W_ih = np.asarray(W_ih, dtype=np.float32)
    b_ih = np.asarray(b_ih, dtype=np.float32)
    W_rzh = np.asarray(W_rzh, dtype=np.float32)
    W_nh = np.asarray(W_nh, dtype=np.float32)
    b_nh = np.asarray(b_nh, dtype=np.float32)

    in_maps = prepare_inputs(x, h, W_ih, b_ih, W_rzh, W_nh, b_nh)
    nc = build_bass()
    res = run_bass_kernel_spmd(nc, in_maps, core_ids=list(range(NCORES)))
    return assemble_output(res.results)


# revision 5
# speedup vs baseline: 1.5033x; 1.0664x over previous
"""Trainium2 Bass kernel for a fused GRU cell — fp8 DoubleRow edition.

Reference computation (B=4096, IN=1024, H=1024, all fp32):
    x_proj = x @ W_ih.T + b_ih            # (B, 3H)
    r_x, z_x, n_x = split(x_proj, 3)
    rz_h = h @ W_rzh.T                    # (B, 2H)
    r = sigmoid(r_x + r_h); z = sigmoid(z_x + z_h)
    n = tanh(n_x + r * (h @ W_nh.T + b_nh))
    out = (1-z)*n + z*h   ==   n + z*(h - n)

Strategy:
  - Data-parallel over batch across 8 NeuronCores (512 rows each);
    weights replicated (packed host-side into PE-friendly tiles).
  - Transposed layout on chip: features on partitions, batch on the free
    dim, so per-feature biases are per-partition ACT activation biases.
  - r/z projections fused into ONE K=2048 contraction by concatenating
    [x;h] and [W_ih[:2H].T; W_rzh.T] host-side.
  - Matmuls in fp8 e4m3 with perf_mode=DoubleRow: each PE instruction
    contracts K=256 (two interleaved fp8 weights per cell), hitting the
    fp8 roofline (~216ns per N=512 instruction).  Weights pre-scaled by
    256 so they sit in e4m3's normal range; the 1/256 is folded into the
    ACT scale (sigmoid/tanh) and b_nh is pre-scaled by 256 so the whole
    n-gate chain stays in the x256 domain until the tanh.
  - Matmul order: r gates g0..g7, then [n-gate tile j | z gate g8+j]
    interleaved so the FINAL matmul group is z-gate g15; after it only
    sigmoid -> z*(h-n) -> +n -> store remain (blend written as
    n + z*(h-n), with d = h-n precomputed under the matmul stream).
  - PSUM accumulation fp32; elementwise intermediates fp16 (2x DVE).
  - DMA issue parallelized: sync = wrz weights + even xh + output
    stores; scalar = brz + odd xh; gpsimd = n-weights/h/biases.  All
    transfers are full tiles (2KB per partition) — sub-sliced strided
    DMAs measured ~5us issue-to-ready.
"""

import numpy as np
import ml_dtypes

import concourse.mybir as mybir
import concourse.tile as tile
from concourse import bacc
from concourse.bass_utils import run_bass_kernel_spmd

B, IN, H = 4096, 1024, 1024
NCORES = 8
BC = B // NCORES          # 512 batch rows per core
P = 128

KO_RZ = (IN + H) // P     # 16 contraction subtiles for the fused r/z matmul
G_RZ = 2 * H // P         # 16 gate tiles (0..7 = r, 8..15 = z)
KO_N = IN // P            # 8
G_N = H // P              # 8
WS = 256.0                # fp8 weight pre-scale

F8 = mybir.dt.float8e4
F16 = mybir.dt.float16
F32 = mybir.dt.float32
AF = mybir.ActivationFunctionType
ALU = mybir.AluOpType
DR = mybir.MatmulPerfMode.DoubleRow
NP8 = ml_dtypes.float8_e4m3


def build_bass():
    """Build the per-core Bass program (identical on all cores)."""
    nc = bacc.Bacc("TRN2", target_bir_lowering=False, debug=False)

    xh_d = nc.dram_tensor("xh", [P, KO_RZ, BC], F8, kind="ExternalInput")
    hf_d = nc.dram_tensor("hf", [P, G_N, BC], F16, kind="ExternalInput")
    wrz_d = nc.dram_tensor("wrz", [G_RZ, P, KO_RZ, P], F8, kind="ExternalInput")
    wn_d = nc.dram_tensor("wn", [G_N, P, 2, KO_N, P], F8, kind="ExternalInput")
    brz_d = nc.dram_tensor("brz", [P, G_RZ], F32, kind="ExternalInput")
    bn_d = nc.dram_tensor("bn", [P, G_N], F32, kind="ExternalInput")
    bnh_d = nc.dram_tensor("bnh", [P, G_N], F32, kind="ExternalInput")
    out_d = nc.dram_tensor("outp", [P, G_N, BC], F16, kind="ExternalOutput")

    with tile.TileContext(nc) as tc:
        with (
            tc.tile_pool(name="const", bufs=1) as cpool,
            tc.tile_pool(name="wrzp", bufs=4) as wrzp,
            tc.tile_pool(name="wnp", bufs=3) as wnp,
            tc.tile_pool(name="rzp", bufs=1) as rzp,
            tc.tile_pool(name="tmp", bufs=4) as tp,
            tc.tile_pool(name="ps_rz", bufs=3, space="PSUM") as pp_rz,
            tc.tile_pool(name="ps_x", bufs=2, space="PSUM") as pp_x,
            tc.tile_pool(name="ps_h", bufs=2, space="PSUM") as pp_h,
            tc.tile_pool(name="ps_w", bufs=1, space="PSUM") as pp_w,
        ):
            # HAM warm-up: the PE clock sits at 1.2 GHz until ~3.4us of
            # sustained activity.  Small N=128 dummy matmuls fill the gap
            # between the engine preamble and the first weight arrival.
            wa = cpool.tile([P, P], F16, tag="warm_l")
            nc.vector.memset(wa[:], 0.0)
            wb = cpool.tile([P, P], F16, tag="warm_r")
            nc.vector.memset(wb[:], 0.0)
            ps_warm = pp_w.tile([P, P], F32, tag="warm_ps")
            for _ in range(14):
                nc.tensor.matmul(ps_warm[:], wa[:], wb[:], start=True, stop=True)

            # First-use DMAs, full tiles only.  sync: w0 + even xh chunks;
            # scalar: brz + odd xh chunks (concurrent issue).
            w0 = wrzp.tile([P, KO_RZ, P], F8, tag="wrz")
            nc.sync.dma_start(out=w0[:], in_=wrz_d[0])
            XH_CH = 4
            xh_chunks = []
            for c in range(KO_RZ // XH_CH):
                t = cpool.tile([P, XH_CH, BC], F8, tag=f"xh{c}", name=f"xh{c}")
                eng = nc.sync if c % 2 == 0 else nc.scalar
                eng.dma_start(out=t[:], in_=xh_d[:, c * XH_CH:(c + 1) * XH_CH, :])
                xh_chunks.append(t)

            def xh_pair(kk):  # [P, 2, BC] fp8 view for DoubleRow pair kk
                c, i = divmod(2 * kk, XH_CH)
                return xh_chunks[c][:, i:i + 2, :]

            brz_sb = cpool.tile([P, G_RZ], F32, tag="brz")
            nc.scalar.dma_start(out=brz_sb[:], in_=brz_d[:])
            bn_sb = cpool.tile([P, G_N], F32, tag="bn")
            nc.gpsimd.dma_start(out=bn_sb[:], in_=bn_d[:])
            bnh_sb = cpool.tile([P, G_N], F32, tag="bnh")
            nc.gpsimd.dma_start(out=bnh_sb[:], in_=bnh_d[:])

            rz_blk = rzp.tile([P, G_RZ, BC], F16, tag="rzblk")
            n_blk = rzp.tile([P, G_N, BC], F16, tag="nblk")
            d_blk = rzp.tile([P, G_N, BC], F16, tag="dblk")
            hf_sb = rzp.tile([P, G_N, BC], F16, tag="hfblk")

            def rz_group(g):
                """DMA gate weights (g>0), run the K=2048 accumulation."""
                if g == 0:
                    w = w0
                else:
                    w = wrzp.tile([P, KO_RZ, P], F8, tag="wrz")
                    nc.sync.dma_start(out=w[:], in_=wrz_d[g])
                if g == 2 or g == 4:
                    half = (g - 2) // 2
                    nc.gpsimd.dma_start(
                        out=hf_sb[:, half * 4:(half + 1) * 4, :],
                        in_=hf_d[:, half * 4:(half + 1) * 4, :],
                    )
                ps = pp_rz.tile([P, BC], F32, tag="psrz")
                for kk in range(KO_RZ // 2):
                    nc.tensor.matmul(
                        ps[:], w[:, 2 * kk:2 * kk + 2, :], xh_pair(kk),
                        start=(kk == 0), stop=(kk == KO_RZ // 2 - 1),
                        perf_mode=DR,
                    )
                return ps

            def sigmoid(g, ps, n_chunks=1):
                CH = BC // n_chunks
                for c in range(n_chunks):
                    s = slice(c * CH, (c + 1) * CH)
                    nc.scalar.activation(
                        rz_blk[:, g, s], ps[:, s], AF.Sigmoid,
                        bias=brz_sb[:, g:g + 1], scale=1.0 / WS,
                    )

            def njob(j):
                """n-gate matmuls + tanh for tile j; leaves n and d = h-n
                in n_blk/d_blk.  Depends on r_j (gate j) but NOT on z."""
                wn_t = wnp.tile([P, 2, KO_N, P], F8, tag="wn")
                nc.gpsimd.dma_start(out=wn_t[:], in_=wn_d[j])
                psh = pp_h.tile([P, BC], F32, tag="psh")
                for kk in range(KO_N // 2):
                    nc.tensor.matmul(
                        psh[:], wn_t[:, 0, 2 * kk:2 * kk + 2, :],
                        xh_pair(KO_N // 2 + kk),
                        start=(kk == 0), stop=(kk == KO_N // 2 - 1),
                        perf_mode=DR,
                    )
                CH = BC // 2
                ts = []
                # t = (psh + 256*b_nh) * r — only needs psh, overlaps psx MMs
                for c in range(2):
                    s = slice(c * CH, (c + 1) * CH)
                    t = tp.tile([P, CH], F32, tag=f"t{c}")
                    nc.vector.scalar_tensor_tensor(
                        t[:], psh[:, s], bnh_sb[:, j:j + 1], rz_blk[:, j, s],
                        op0=ALU.add, op1=ALU.mult,
                    )
                    ts.append(t)
                psx = pp_x.tile([P, BC], F32, tag="psx")
                for kk in range(KO_N // 2):
                    nc.tensor.matmul(
                        psx[:], wn_t[:, 1, 2 * kk:2 * kk + 2, :], xh_pair(kk),
                        start=(kk == 0), stop=(kk == KO_N // 2 - 1),
                        perf_mode=DR,
                    )
                for c in range(2):
                    s = slice(c * CH, (c + 1) * CH)
                    t = ts[c]
                    nc.vector.tensor_add(out=t[:], in0=t[:], in1=psx[:, s])
                    nc.scalar.activation(
                        n_blk[:, j, s], t[:], AF.Tanh, bias=bn_sb[:, j:j + 1],
                        scale=1.0 / WS,
                    )
                    nc.vector.tensor_sub(
                        out=d_blk[:, j, s], in0=hf_sb[:, j, s],
                        in1=n_blk[:, j, s],
                    )

            def final(jz, n_chunks):
                """out = n + z*(h-n); only needs sigmoid(8+jz) and d/n."""
                o = tp.tile([P, BC], F16, tag="o")
                CH = BC // n_chunks
                for c in range(n_chunks):
                    s = slice(c * CH, (c + 1) * CH)
                    m = tp.tile([P, CH], F16, tag=f"m{c}")
                    nc.vector.tensor_mul(
                        out=m[:], in0=rz_blk[:, G_N + jz, s], in1=d_blk[:, jz, s]
                    )
                    nc.vector.tensor_add(
                        out=o[:, s], in0=n_blk[:, jz, s], in1=m[:]
                    )
                    if n_chunks > 1:
                        nc.sync.dma_start(out=out_d[:, jz, s], in_=o[:, s])
                if n_chunks == 1:
                    nc.sync.dma_start(out=out_d[:, jz, :], in_=o[:])

            # r gates
            for g in range(G_N):
                sigmoid(g, rz_group(g))
            # pipelined: n-tile j runs between z gates so the last matmul
            # group is z gate g15
            njob(0)
            for jz in range(G_N):
                ps = rz_group(G_N + jz)
                last = jz == G_N - 1
                sigmoid(G_N + jz, ps, n_chunks=2 if last else 1)
                final(jz, n_chunks=2 if last else 1)
                if not last:
                    njob(jz + 1)

    nc.compile()
    return nc


def q8(a):
    return np.clip(a, -240.0, 240.0).astype(NP8)


def prepare_inputs(x, h, W_ih, b_ih, W_rzh, W_nh, b_nh):
    """Host-side packing: shard batch, transpose/concat/quantize weights."""
    # Fused r/z weight: (IN+H, 2H) -> [g, p, ko, mi] tile-major, fp8 x256
    wrz_cat = q8(np.concatenate([W_ih[: 2 * H].T, W_rzh.T], axis=0) * WS)
    wrz = np.ascontiguousarray(
        wrz_cat.reshape(KO_RZ, P, G_RZ, P).transpose(2, 1, 0, 3)
    )
    wnx = q8(W_ih[2 * H:].T * WS).reshape(KO_N, P, G_N, P).transpose(2, 1, 0, 3)
    wnh = q8(W_nh.T * WS).reshape(KO_N, P, G_N, P).transpose(2, 1, 0, 3)
    # [G, P, 2, KO, P]: slot 0 = W_nh, slot 1 = W_nx
    wn = np.ascontiguousarray(np.stack([wnh, wnx], axis=2))
    brz = np.ascontiguousarray(b_ih[: 2 * H].reshape(G_RZ, P).T).astype(np.float32)
    bn = np.ascontiguousarray(b_ih[2 * H:].reshape(G_N, P).T).astype(np.float32)
    bnh = np.ascontiguousarray((b_nh * WS).reshape(G_N, P).T).astype(np.float32)

    xh_catT = q8(np.concatenate([x.T, h.T], axis=0))           # (2048, B) fp8
    hT = np.ascontiguousarray(h.T.astype(np.float16))          # (1024, B)

    in_maps = []
    for c in range(NCORES):
        cols = slice(c * BC, (c + 1) * BC)
        xh_c = np.ascontiguousarray(
            xh_catT[:, cols].reshape(KO_RZ, P, BC).transpose(1, 0, 2)
        )
        hf_c = np.ascontiguousarray(
            hT[:, cols].reshape(G_N, P, BC).transpose(1, 0, 2)
        )
        in_maps.append(
            {
                "xh": xh_c,
                "hf": hf_c,
                "wrz": wrz,
                "wn": wn,
                "brz": brz,
                "bn": bn,
                "bnh": bnh,
            }
        )
    return in_maps


def assemble_output(results):
    """results: list of per-core dicts with 'outp' [P, G_N, BC] fp16."""
    parts = []
    for c in range(NCORES):
        oc = np.asarray(results[c]["outp"]).astype(np.float32)
        ocT = oc.transpose(1, 0, 2).reshape(H, BC)    # features x batch
        parts.append(np.ascontiguousarray(ocT.T))     # batch x features
    return np.concatenate(parts, axis=0).astype(np.float32)


def kernel(x, h, W_ih, b_ih, W_rzh, W_nh, b_nh):
    x = np.asarray(x, dtype=np.float32)
    h = np.asarray(h, dtype=np.float32)
    W_ih = np.asarray(W_ih, dtype=np.float32)
    b_ih = np.asarray(b_ih, dtype=np.float32)
    W_rzh = np.asarray(W_rzh, dtype=np.float32)
    W_nh = np.asarray(W_nh, dtype=np.float32)
    b_nh = np.asarray(b_nh, dtype=np.float32)

    in_maps = prepare_inputs(x, h, W_ih, b_ih, W_rzh, W_nh, b_nh)
    nc = build_bass()
    res = run_bass_kernel_spmd(nc, in_maps, core_ids=list(range(NCORES)))
    return assemble_output(res.results)
